# revision 2
# baseline (speedup 1.0000x reference)
"""Trainium2 Bass kernel for CLIP attention pooling.

Reference computation (N=4096, D=1024, fp32):
    q = x @ Wq.T + bq
    k = x @ Wk.T + bk
    attn = softmax(q @ k.T, axis=-1)
    out = attn @ x

Math notes:
  * scores = q @ k.T = q @ Wk @ x.T + (q.bk) 1^T. The (q.bk) term is
    constant along the softmax axis, so bk never needs to be computed.
  * q @ Wk = x @ (Wq.T @ Wk) + bq @ Wk: both projections fold into one
    matrix M = Wq.T @ Wk and a row c = bq @ Wk (host-precomputed).
  * Per core (512 query rows):
        tT = M^T . xs^T + c          [D, 512]   (transposed layout)
        S  = t . x^T                 [512, 4096]
        P  = softmax(S)              (online, running-max)
        out = P @ x                  [512, 1024]

Schedule (v2):
  * phase A: M chunks stream on the sync HWDGE ring, xs chunks on the
    scalar ring (independent trigger FIFOs); e-outer over 8 PSUM banks;
    the bias row enters via a K=1 (c x ones) matmul per bank.
  * phase B: x^T streams in 512-column chunks; chunks 0-1 are split
    into 256-column halves (full fp32r rate needs moving>=256) so the
    first matmuls start as soon as 1MB lands. Softmax is ONLINE: per
    (i, chunk) a running negated max is maintained on DVE straight out
    of PSUM, and ACT applies exp(PSUM - runmax) directly into bf16 E
    with accum_out collecting per-chunk partial sums. No S buffer.
  * after the last chunk: c_k = exp(m_k - m_final) per chunk, Z =
    sum_k z_k c_k, g_k = c_k / Z. The g_k become 32 per-(i,chunk)
    DIAGONAL matrices (bf16); phase C "transposes" are plain matmuls
    E_tile @ diag(g) so the softmax normalization and the running-max
    corrections ride the mandatory transpose for free.
  * phase C: x (bf16) is fully resident in SBUF (loaded on the sync
    ring behind the phase-B stream, reusing the phase-A weight pool
    address space). Pass 0 covers i-tiles {0,1,2} (6 PSUM accumulators
    + 2 transpose banks), pass 1 covers {3}, so after the final matmul
    only 512KB of output remains to copy+DMA. Output copy/DMA pairs
    alternate DVE/ACT engines and sync/scalar DMA rings.
"""

import os
from contextlib import ExitStack

import numpy as np
import ml_dtypes

import concourse.bass as bass
import concourse.mybir as mybir
import concourse.tile as tile
from concourse import bacc
from concourse.bass_utils import run_bass_kernel_spmd
from concourse.masks import make_identity

N, D = 4096, 1024
NCORES = 8
R = N // NCORES  # 512 query rows per core
PT = 128  # partition tile
EC = D // PT  # 8 contraction chunks of the model dim
IT = R // PT  # 4 query tiles per core
JC = N // 512  # 8 key chunks of 512
JT = N // PT  # 32 key tiles of 128
NSPLIT = 2  # leading key chunks streamed/computed as 256-wide halves

F32 = mybir.dt.float32
F32R = mybir.dt.float32r
BF16 = mybir.dt.bfloat16
AX = mybir.AxisListType
AF = mybir.ActivationFunctionType
ALU = mybir.AluOpType


def _emit(nc: bass.Bass, tc: tile.TileContext, aps: dict):
    xTb, xTbh, xTs, mw, cw, ones, xb, out = (
        aps["xTb"], aps["xTbh"], aps["xTs"], aps["mw"], aps["cw"],
        aps["ones"], aps["xb"], aps["out"],
    )

    with ExitStack() as big:
        persist = big.enter_context(tc.tile_pool(name="persist", bufs=1))

        ident = persist.tile([PT, PT], BF16)
        make_identity(nc, ident)
        c_sb = persist.tile([1, D], F32R)
        ones_sb = persist.tile([1, R], F32R)

        tT_sb = persist.tile([PT, EC, R], F32R)
        E_bf = [persist.tile([PT, N], BF16, name=f"E{i}") for i in range(IT)]
        nmk = [persist.tile([PT, JC], F32, name=f"nmk{i}") for i in range(IT)]
        zpart = [persist.tile([PT, JC], F32, name=f"zp{i}") for i in range(IT)]
        ck = [persist.tile([PT, JC], F32, name=f"ck{i}") for i in range(IT)]
        gk = [persist.tile([PT, JC], F32, name=f"gk{i}") for i in range(IT)]
        zsum = [persist.tile([PT, 1], F32, name=f"z{i}") for i in range(IT)]
        rz = [persist.tile([PT, 1], F32, name=f"rz{i}") for i in range(IT)]
        diag = persist.tile([PT, IT, JC, PT], BF16)

        # opened before wpool so its addresses never overlap the weights;
        # the early stream triggers can then issue during phase A.
        xtpool = big.enter_context(tc.tile_pool(name="xtpool", bufs=3))

        # ---- Phase A: tT = M^T.xs^T + c  (transposed layout)
        with ExitStack() as pha:
            wpool = pha.enter_context(tc.tile_pool(name="wpool", bufs=1))
            apsum = pha.enter_context(tc.tile_pool(name="apsum", bufs=1, space="PSUM"))

            m_sb = wpool.tile([PT, EC, D], F32R)
            xts_sb = wpool.tile([PT, EC, R], F32R)

            m_r = mw.rearrange("(t p) d -> p t d", p=PT)
            xTs_r = xTs.rearrange("(t p) i -> p t i", p=PT)
            # M rides the sync HWDGE ring, xs + bias the scalar ring: the
            # trigger FIFOs are independent and the SDMA engines round-robin
            # between them.
            nc.sync.dma_start(m_sb[:, 0, 0:PT], m_r[:, 0, 0:PT])
            nc.scalar.dma_start(xts_sb[:, 0, :], xTs_r[:, 0, :])
            nc.sync.dma_start(m_sb[:, 0, PT:D], m_r[:, 0, PT:D])
            nc.scalar.dma_start(xts_sb[:, 1, :], xTs_r[:, 1, :])
            nc.scalar.dma_start(c_sb, cw)
            nc.scalar.dma_start(ones_sb, ones)
            for e in range(1, EC):
                nc.sync.dma_start(m_sb[:, e, :], m_r[:, e, :])
            for e in range(2, EC):
                nc.scalar.dma_start(xts_sb[:, e, :], xTs_r[:, e, :])

            tps = [
                apsum.tile([PT, R], F32, tag=f"tp{d}", name=f"tp{d}")
                for d in range(EC)
            ]
            for e in range(EC):
                for d in range(EC):
                    nc.tensor.matmul(
                        tps[d],
                        m_sb[:, e, d * PT : (d + 1) * PT],
                        xts_sb[:, e, :],
                        start=(e == 0),
                        stop=False,
                    )
            for d in range(EC):
                # bias row: tT[d_block, :] += c[d_block] (x) ones
                nc.tensor.matmul(
                    tps[d],
                    c_sb[:, d * PT : (d + 1) * PT],
                    ones_sb,
                    start=False,
                    stop=True,
                )
                if d % 2 == 0:
                    nc.vector.tensor_copy(tT_sb[:, d, :], tps[d])
                else:
                    nc.scalar.activation(tT_sb[:, d, :], tps[d], func=AF.Copy)

        # ---- Phase B: S chunks in PSUM + online softmax straight to E.
        with ExitStack() as phb:
            spsum = phb.enter_context(tc.tile_pool(name="spsum", bufs=5, space="PSUM"))
            tmpool = phb.enter_context(tc.tile_pool(name="tmpool", bufs=4))
            for j in range(JC):
                xtj = xtpool.tile([PT, EC, 512], F32R, tag="xtj", name="xtj")
                if j < NSPLIT:
                    nc.sync.dma_start(xtj[:, :, 0:256], xTbh[2 * j])
                    nc.sync.dma_start(xtj[:, :, 256:512], xTbh[2 * j + 1])
                else:
                    nc.sync.dma_start(xtj, xTb[j])

                pss = []
                if j < NSPLIT:
                    # half-outer so all i-tiles chew on half 0 while half 1
                    # is still in flight.
                    for i in range(IT):
                        pss.append(spsum.tile([PT, 512], F32, tag="Sp", name="Sp"))
                    for h in range(2):
                        for i in range(IT):
                            for d in range(EC):
                                nc.tensor.matmul(
                                    pss[i][:, h * 256 : (h + 1) * 256],
                                    tT_sb[:, d, i * PT : (i + 1) * PT],
                                    xtj[:, d, h * 256 : (h + 1) * 256],
                                    start=(d == 0),
                                    stop=(d == EC - 1),
                                    skip_group_check=True,
                                )
                else:
                    for i in range(IT):
                        ps = spsum.tile([PT, 512], F32, tag="Sp", name="Sp")
                        pss.append(ps)
                        for d in range(EC):
                            nc.tensor.matmul(
                                ps,
                                tT_sb[:, d, i * PT : (i + 1) * PT],
                                xtj[:, d, :],
                                start=(d == 0),
                                stop=(d == EC - 1),
                            )
                for i in range(IT):
                    ps = pss[i]
                    if j == 0:
                        nc.vector.reduce_max(
                            out=nmk[i][:, 0:1], in_=ps, axis=AX.X, negate=True
                        )
                    else:
                        tm = tmpool.tile([PT, 1], F32, tag="tm", name="tm")
                        nc.vector.reduce_max(out=tm, in_=ps, axis=AX.X, negate=True)
                        nc.vector.tensor_tensor(
                            out=nmk[i][:, j : j + 1],
                            in0=nmk[i][:, j - 1 : j],
                            in1=tm,
                            op=ALU.min,
                        )
                    nc.scalar.activation(
                        out=E_bf[i][:, j * 512 : (j + 1) * 512],
                        in_=ps,
                        func=AF.Exp,
                        bias=nmk[i][:, j : j + 1],
                        scale=1.0,
                        accum_out=zpart[i][:, j : j + 1],
                    )

            # finalize: c_k = exp(m_k - m_last), Z = sum z_k c_k, g = c_k/Z
            for i in range(IT):
                nc.scalar.activation(
                    out=ck[i],
                    in_=nmk[i],
                    func=AF.Exp,
                    bias=nmk[i][:, JC - 1 : JC],
                    scale=-1.0,
                )
                nc.vector.tensor_tensor(
                    out=gk[i], in0=zpart[i], in1=ck[i], op=ALU.mult
                )
                nc.vector.reduce_sum(out=zsum[i], in_=gk[i], axis=AX.X)
                nc.vector.reciprocal(rz[i], zsum[i])
                nc.vector.tensor_scalar_mul(gk[i], ck[i], rz[i])
            for k in range(JC):
                for i in range(IT):
                    nc.vector.tensor_scalar_mul(
                        diag[:, i, k, :], ident, gk[i][:, k : k + 1]
                    )

        # ---- Phase C: out = P @ x with x fully resident in SBUF.
        # xb reuses the phase-A weight pool's address range; its triggers sit
        # on the sync ring behind the phase-B stream.
        xbpool = big.enter_context(tc.tile_pool(name="xbpool", bufs=1))
        xb_sb = xbpool.tile([PT, JT, D], BF16)
        xb_r = xb.rearrange("(t p) d -> p t d", p=PT)
        for g in range(8):
            nc.sync.dma_start(
                xb_sb[:, 4 * g : 4 * g + 4, :], xb_r[:, 4 * g : 4 * g + 4, :]
            )
        etpool = big.enter_context(tc.tile_pool(name="etpool", bufs=4))
        ocopy = big.enter_context(tc.tile_pool(name="ocopy", bufs=3))

        LOOK = 2
        for pi, ii in enumerate(((0, 1, 2), (3,))):
            W = len(ii) * PT
            with ExitStack() as phc:
                opsum = phc.enter_context(
                    tc.tile_pool(name=f"opsum{pi}", bufs=1, space="PSUM")
                )
                tpsum = phc.enter_context(
                    tc.tile_pool(name=f"tpsum{pi}", bufs=2, space="PSUM")
                )
                oacc = {
                    (i, dn): opsum.tile(
                        [PT, 512], F32, tag=f"o{i}_{dn}", name=f"o{i}_{dn}"
                    )
                    for i in ii
                    for dn in range(2)
                }
                ets = {}
                for jtv in range(JT + LOOK):
                    if jtv < JT:
                        jt = jtv
                        # "transpose" = E_tile.T @ diag(g): per-row softmax
                        # scale applied for free by the mandatory transpose.
                        pst = tpsum.tile([PT, W], F32, tag="tp", name="pst")
                        for kp, i in enumerate(ii):
                            nc.tensor.matmul(
                                pst[:, kp * PT : (kp + 1) * PT],
                                E_bf[i][:, jt * PT : (jt + 1) * PT],
                                diag[:, i, jt // 4, :],
                                start=True,
                                stop=True,
                                skip_group_check=True,
                            )
                        et = etpool.tile([PT, W], BF16, tag=f"et{pi}", name="et")
                        nc.vector.tensor_copy(et, pst)
                        ets[jt % 4] = et
                    if jtv >= LOOK:
                        jt = jtv - LOOK
                        for kp, i in enumerate(ii):
                            for dn in range(2):
                                nc.tensor.matmul(
                                    oacc[(i, dn)],
                                    ets[jt % 4][:, kp * PT : (kp + 1) * PT],
                                    xb_sb[:, jt, dn * 512 : (dn + 1) * 512],
                                    start=(jt == 0),
                                    stop=(jt == JT - 1),
                                )
                for kp, i in enumerate(ii):
                    for dn in range(2):
                        ot = ocopy.tile([PT, 512], F32, tag="ot", name="ot")
                        if dn == 0:
                            nc.vector.tensor_copy(ot, oacc[(i, dn)])
                            nc.sync.dma_start(
                                out[i * PT : (i + 1) * PT, 0:512], ot
                            )
                        else:
                            nc.scalar.activation(ot, oacc[(i, dn)], func=AF.Copy)
                            nc.scalar.dma_start(
                                out[i * PT : (i + 1) * PT, 512:1024], ot
                            )


def build():
    nc = bacc.Bacc(
        "TRN2",
        target_bir_lowering=False,
        debug=False,
        enable_asserts=False,
        num_devices=NCORES,
    )
    aps = {
        "xTb": nc.dram_tensor("xTb", [JC, PT, EC, 512], F32R, kind="ExternalInput").ap(),
        "xTbh": nc.dram_tensor(
            "xTbh", [2 * NSPLIT, PT, EC, 256], F32R, kind="ExternalInput"
        ).ap(),
        "xTs": nc.dram_tensor("xTs", [D, R], F32R, kind="ExternalInput").ap(),
        "mw": nc.dram_tensor("mw", [D, D], F32R, kind="ExternalInput").ap(),
        "cw": nc.dram_tensor("cw", [1, D], F32R, kind="ExternalInput").ap(),
        "ones": nc.dram_tensor("ones", [1, R], F32R, kind="ExternalInput").ap(),
        "xb": nc.dram_tensor("xb", [N, D], BF16, kind="ExternalInput").ap(),
        "out": nc.dram_tensor("out", [R, D], F32, kind="ExternalOutput").ap(),
    }
    with tile.TileContext(nc) as tc:
        _emit(nc, tc, aps)
    nc.compile()
    return nc


_NC_CACHE = None
LAST_RESULTS = None


def _get_nc():
    global _NC_CACHE
    if _NC_CACHE is None:
        _NC_CACHE = build()
    return _NC_CACHE


def make_in_maps(x, Wq, bq, Wk):
    x = np.ascontiguousarray(np.asarray(x, dtype=np.float32))
    xT = np.ascontiguousarray(x.T)
    # xTb[j, p, e, n] = xT[e*128 + p, j*512 + n]: per-(j,p) contiguous 16KB
    # blocks so the phase-B stream DMAs at full descriptor size.
    xTb = np.ascontiguousarray(
        xT.reshape(EC, PT, JC, 512).transpose(2, 1, 0, 3)
    )
    # first NSPLIT chunks also staged as 256-wide halves (8KB rows).
    xTbh = np.ascontiguousarray(
        xT[:, : NSPLIT * 512].reshape(EC, PT, 2 * NSPLIT, 256).transpose(2, 1, 0, 3)
    )
    wk64 = np.asarray(Wk, dtype=np.float64)
    mw = np.ascontiguousarray(
        (np.asarray(Wq, dtype=np.float64).T @ wk64).astype(np.float32)
    )
    cw = np.ascontiguousarray(
        (np.asarray(bq, dtype=np.float64) @ wk64).astype(np.float32).reshape(1, D)
    )
    ones_arr = np.ones((1, R), dtype=np.float32)
    xb = x.astype(ml_dtypes.bfloat16)
    in_maps = []
    for c in range(NCORES):
        in_maps.append(
            {
                "xTb": xTb,
                "xTbh": xTbh,
                "xTs": np.ascontiguousarray(xT[:, c * R : (c + 1) * R]),
                "mw": mw,
                "cw": cw,
                "ones": ones_arr,
                "xb": xb,
            }
        )
    return in_maps


def kernel(x, Wq, bq, Wk, bk):
    # bk only shifts each score row by a constant, which softmax cancels.
    del bk
    in_maps = make_in_maps(x, Wq, bq, Wk)
    nc = _get_nc()
    kwargs = {}
    if os.environ.get("K_TRACE_DIR"):
        import tempfile

        kwargs["tmpdir"] = tempfile.mkdtemp(dir=os.environ["K_TRACE_DIR"])
    res = run_bass_kernel_spmd(nc, in_maps, core_ids=list(range(NCORES)), **kwargs)
    global LAST_RESULTS
    LAST_RESULTS = res
    return np.concatenate(
        [np.asarray(res.results[c]["out"], dtype=np.float32) for c in range(NCORES)],
        axis=0,
    )
